# revision 1
# baseline (speedup 1.0000x reference)
"""Trainium2 Bass kernel for a 2-layer GCN encoder (AssemblyQueryEncoder).

Reference computation (PyG-style GCNConv x2 + global mean pool + linear + L2norm):
    h1 = relu(gcnconv(x, W1, b1));  h2 = relu(gcnconv(h1, W2, b2))
    g  = segment_mean(h2, batch) @ Wl + bl;  out = g / max(||g||_2, eps)

Distribution over 8 NeuronCores:
  - Nodes sharded contiguously (5120 padded/core); each core owns the incoming
    edges of its nodes (destination partitioning).
  - Per layer: local transform h = x @ W (bf16); the AllGather of h is SPLIT
    into NQ quarter-collectives (each gathers a quarter of every rank's shard
    into its own table tensor) so aggregation of quarter-q edges overlaps the
    remaining quarters' communication.  Quarter tables have <= 10240 rows so
    dma_gather's int16 indices cover them directly; 1024-index gathers are
    round-robined over the 4 SWDGE queues.
  - Aggregation is per-destination-block selection-matrix matmuls accumulated
    in PSUM; selection matrices (edge norms scattered into one-hot columns)
    are host-precomputed bf16 and streamed from DRAM via HWDGE.  The
    self-loop term rides a per-block diagonal matmul and the bias a rank-1
    (K=1) matmul, so the epilogue is a single Relu out of PSUM.
  - Pooled per-graph sums (1/count folded into the pooling matrix) are
    AllReduced ([128,64]); final linear + L2 norm computed redundantly in f32.
"""

import sys

sys.path.insert(0, "/opt/trn_rl_repo")

import numpy as np

P = 128  # partitions


def _cdiv(a, b):
    return (a + b - 1) // b


def _cdiv_arr(a, b):
    return (a + b - 1) // b


class GCNConfig:
    def __init__(self, n_nodes=40000, n_graphs=64, d_in=128, d_hid=128, d_out=64,
                 n_cores=8, chunk_tiles=8):
        self.n_nodes = n_nodes
        self.n_graphs = n_graphs
        self.d_in = d_in
        self.d_hid = d_hid
        self.d_out = d_out
        self.n_cores = n_cores
        self.chunk_tiles = chunk_tiles
        self.nloc = _cdiv(n_nodes, n_cores * P) * P  # padded nodes per core
        self.npad = self.nloc * n_cores
        self.nblk = self.nloc // P  # 128-node blocks per core
        # number of AllGather splits; quarter tables must stay int16-addressable
        self.nq = 4 if (self.nblk % 4 == 0
                        and (self.nloc // 4) * n_cores <= 32768) else 1
        assert (self.nloc // self.nq) * n_cores <= 32768


def _wrap_idx(flat):
    """dma_gather index layout: element i -> [i % 16, i // 16], x8 partitions."""
    n = flat.shape[0]
    assert n % 16 == 0
    arr = np.zeros((16, n // 16), np.int16)
    arr[np.arange(n) % 16, np.arange(n) // 16] = flat
    return np.tile(arr, (8, 1))


def preprocess(cfg, x, edge_index, batch):
    """Host-side index preprocessing. Edges are grouped per core by destination
    block and split into nq streams by source quarter; each (block, stream)
    list is padded to a tile multiple shared by all cores. Selection matrices
    are materialized dense (bf16) for streaming."""
    import ml_dtypes
    bfd = ml_dtypes.bfloat16

    n, nc_ = cfg.n_nodes, cfg.n_cores
    nq, qs = cfg.nq, cfg.nloc // cfg.nq
    src_a = np.asarray(edge_index[0], dtype=np.int64)
    dst_a = np.asarray(edge_index[1], dtype=np.int64)
    batch = np.asarray(batch, dtype=np.int64)

    deg = np.bincount(dst_a, minlength=n).astype(np.float64) + 1.0
    dinv = 1.0 / np.sqrt(deg)
    nrm_a = (dinv[src_a] * dinv[dst_a]).astype(np.float32)
    dinv2 = (dinv * dinv).astype(np.float32)

    # source quarter + row within that quarter's gathered table
    q_a = (src_a % cfg.nloc) // qs
    row_a = (src_a // cfg.nloc) * qs + (src_a % qs)

    order = np.lexsort((dst_a, q_a))
    src_q = q_a[order]
    dst_s = dst_a[order]
    row_s = row_a[order]
    nrm_s = nrm_a[order]
    qstart = np.searchsorted(src_q, np.arange(nq + 1))

    nblk_g = cfg.npad // P
    res = {"T": [], "ttot": []}
    for q in range(nq):
        lo_, hi_ = qstart[q], qstart[q + 1]
        s_r = row_s[lo_:hi_]
        s_d = dst_s[lo_:hi_]
        s_n = nrm_s[lo_:hi_]
        blk = s_d // P
        counts = np.bincount(blk, minlength=nblk_g).reshape(nc_, cfg.nblk)
        T = _cdiv_arr(counts.max(axis=0), P).astype(np.int64)
        ttot = max(int(T.sum()), 1)
        tstart = np.concatenate([[0], np.cumsum(T)]).astype(np.int64)
        bstart = np.concatenate(
            [[0], np.cumsum(np.bincount(blk, minlength=nblk_g))]).astype(np.int64)
        gidx = np.zeros((nc_, P, ttot), np.int16)
        msel = np.zeros((nc_, P, ttot, P), bfd)
        for c in range(nc_):
            for b in range(cfg.nblk):
                gb = c * cfg.nblk + b
                e0, e1 = bstart[gb], bstart[gb + 1]
                m = e1 - e0
                if m == 0:
                    continue
                jj = np.arange(m)
                pp, tt = jj % P, tstart[b] + jj // P
                gidx[c, pp, tt] = s_r[e0:e1]
                msel[c, pp, tt, (s_d[e0:e1] % P)] = s_n[e0:e1].astype(bfd)
        widx = np.stack([_wrap_idx(gidx[c].T.reshape(-1)) for c in range(nc_)])
        res[f"gidx{q}"] = widx
        res[f"msel{q}"] = msel.reshape(nc_, P, ttot * P)
        res["T"].append(T)
        res["ttot"].append(ttot)

    # per-block diagonal self-loop matrices [P, nblk*P] bf16
    dg = np.zeros((nc_, P, cfg.nblk * P), bfd)
    d2pad = np.zeros(cfg.npad, np.float32)
    d2pad[:n] = dinv2
    for c in range(nc_):
        for b in range(cfg.nblk):
            base = c * cfg.nloc + b * P
            dg[c, np.arange(P), b * P + np.arange(P)] = d2pad[base:base + P].astype(bfd)

    # x transposed per core, padded, bf16
    xT = np.zeros((nc_, cfg.d_in, cfg.nloc), bfd)
    xf = np.asarray(x, dtype=np.float32)
    for c in range(nc_):
        lo2, hi2 = c * cfg.nloc, min((c + 1) * cfg.nloc, n)
        if hi2 > lo2:
            xT[c, :, : hi2 - lo2] = xf[lo2:hi2].T.astype(bfd)

    # pooling matrix with 1/count folded in, block-major [P, nblk*G]
    g_ = cfg.n_graphs
    cnt = np.maximum(np.bincount(batch, minlength=g_).astype(np.float32), 1.0)
    pm = np.zeros((nc_, P, cfg.nblk * g_), np.float32)
    for c in range(nc_):
        for b in range(cfg.nblk):
            base = c * cfg.nloc + b * P
            hi2 = min(base + P, n)
            if hi2 <= base:
                continue
            rows = np.arange(hi2 - base)
            gg = batch[base:hi2]
            pm[c, rows, b * g_ + gg] = 1.0 / cnt[gg]

    res.update(xT=xT, pm=pm, dg=dg)
    return res


def build(cfg, Ts, ttots):
    """Build the SPMD Bass graph (same program for all cores)."""
    import concourse.mybir as mybir
    import concourse.tile as tile
    from concourse import bacc
    from concourse.masks import make_identity

    f32 = mybir.dt.float32
    bf = mybir.dt.bfloat16
    i16 = mybir.dt.int16
    AF = mybir.ActivationFunctionType
    ALU = mybir.AluOpType

    nc_ = cfg.n_cores
    nblk = cfg.nblk
    nloc = cfg.nloc
    nq, qs = cfg.nq, cfg.nloc // cfg.nq
    qblk = nblk // nq
    dh = cfg.d_hid
    do = cfg.d_out
    g_ = cfg.n_graphs
    rg = [list(range(nc_))]
    chunk = cfg.chunk_tiles

    nc = bacc.Bacc("TRN2", target_bir_lowering=False, debug=False,
                   num_devices=nc_, num_swdge_queues=4)

    # ---- parameters ----
    xT_p = nc.declare_dram_parameter("xT", [cfg.d_in, nloc], bf, isOutput=False)
    gidx_p, msel_p = [], []
    for q in range(nq):
        gidx_p.append(nc.declare_dram_parameter(
            f"gidx{q}", [P, ttots[q] * 8], i16, isOutput=False))
        msel_p.append(nc.declare_dram_parameter(
            f"msel{q}", [P, ttots[q] * P], bf, isOutput=False))
    dg_p = nc.declare_dram_parameter("dg", [P, nblk * P], bf, isOutput=False)
    pm_p = nc.declare_dram_parameter("pm", [P, nblk * g_], f32, isOutput=False)
    w1_p = nc.declare_dram_parameter("W1", [cfg.d_in, dh], bf, isOutput=False)
    w2_p = nc.declare_dram_parameter("W2", [dh, dh], bf, isOutput=False)
    wl_p = nc.declare_dram_parameter("Wl", [dh, do], f32, isOutput=False)
    b1_p = nc.declare_dram_parameter("b1", [1, dh], bf, isOutput=False)
    b2_p = nc.declare_dram_parameter("b2", [1, dh], bf, isOutput=False)
    bl_p = nc.declare_dram_parameter("bl", [1, do], f32, isOutput=False)
    out_p = nc.declare_dram_parameter("out", [g_, do], f32, isOutput=True)

    # ---- internal DRAM ----
    agin = [nc.dram_tensor(f"agin{l}", [nloc, dh], bf) for l in (1, 2)]
    tables = [[nc.dram_tensor(f"table{l}_{q}", [nc_ * qs, dh], bf,
                              addr_space="Shared") for q in range(nq)]
              for l in (1, 2)]
    arin = nc.dram_tensor("arin", [dh, g_], f32)
    arout = nc.dram_tensor("arout", [dh, g_], f32, addr_space="Shared")

    tstarts = [np.concatenate([[0], np.cumsum(T)]).astype(np.int64) for T in Ts]

    with tile.TileContext(nc) as tc:
        with (
            tc.tile_pool(name="const", bufs=1) as cpool,
            tc.tile_pool(name="big", bufs=1) as bigpool,
            tc.tile_pool(name="gat", bufs=12) as gpool,
            tc.tile_pool(name="m", bufs=6) as mpool,
            tc.tile_pool(name="small", bufs=2) as spool,
            tc.tile_pool(name="psum", bufs=3, space="PSUM") as pspool,
            tc.tile_pool(name="psum1", bufs=1, space="PSUM") as pspool1,
        ):
            # ---- constants ----
            w1_sb = cpool.tile([cfg.d_in, dh], bf)
            w2_sb = cpool.tile([dh, dh], bf)
            wl_sb = cpool.tile([dh, do], f32)
            b1_sb = cpool.tile([1, dh], bf)
            b2_sb = cpool.tile([1, dh], bf)
            bl_sb = cpool.tile([1, do], f32)
            ones_sb = cpool.tile([1, P], bf)
            onesf_sb = cpool.tile([1, P], f32)
            ident_sb = cpool.tile([P, P], f32)
            nc.sync.dma_start(w1_sb[:], w1_p[:])
            nc.sync.dma_start(w2_sb[:], w2_p[:])
            nc.sync.dma_start(wl_sb[:], wl_p[:])
            nc.sync.dma_start(b1_sb[:], b1_p[:])
            nc.sync.dma_start(b2_sb[:], b2_p[:])
            nc.sync.dma_start(bl_sb[:], bl_p[:])
            nc.gpsimd.memset(ones_sb[:], 1.0)
            nc.gpsimd.memset(onesf_sb[:], 1.0)
            make_identity(nc, ident_sb[:])

            xT_sb = bigpool.tile([cfg.d_in, nloc], bf, tag="lhsT")
            gidx_sb = []
            for q in range(nq):
                t = bigpool.tile([P, ttots[q] * 8], i16, tag=f"gidx{q}")
                nc.sync.dma_start(t[:], gidx_p[q][:])
                gidx_sb.append(t)
            dg_sb = bigpool.tile([P, nblk * P], bf)
            pm_sb = bigpool.tile([P, nblk * g_], f32)
            nc.sync.dma_start(xT_sb[:], xT_p[:])
            nc.sync.dma_start(dg_sb[:], dg_p[:])
            nc.sync.dma_start(pm_sb[:], pm_p[:])

            hpre_sb = bigpool.tile([P, nloc], bf)

            def bsl(b, w=P):
                return slice(b * w, (b + 1) * w)

            def transform(lhsT_sb, w_sb, layer):
                ag = agin[layer]
                for q in range(nq):
                    for b in range(q * qblk, (q + 1) * qblk):
                        ps = pspool.tile([P, dh], f32, tag="pst")
                        nc.tensor.matmul(ps[:], lhsT_sb[:, bsl(b)], w_sb[:],
                                         start=True, stop=True)
                        nc.vector.tensor_copy(hpre_sb[:, bsl(b)], ps[:])
                    nc.sync.dma_start(
                        ag[q * qs:(q + 1) * qs, :].rearrange(
                            "(b p) f -> p b f", p=P),
                        hpre_sb[:, q * qblk * dh:(q + 1) * qblk * dh].rearrange(
                            "p (b f) -> p b f", f=dh))
                    nc.gpsimd.collective_compute(
                        "AllGather", mybir.AluOpType.bypass, replica_groups=rg,
                        ins=[ag[q * qs:(q + 1) * qs, :]],
                        outs=[tables[layer][q][:]])

            def aggregate(layer, b_sb, hout_sb):
                streams = []
                for q in range(nq):
                    streams.append(dict(
                        tstart=tstarts[q], ttot=ttots[q], gidx=gidx_sb[q],
                        mp=msel_p[q], view=tables[layer][q][:],
                        gcur=None, gc0=-1, mcur=None, q=q))

                def fetch(st, t):
                    c0 = (t // chunk) * chunk
                    if st["gc0"] != c0:
                        k = min(chunk, st["ttot"] - c0)
                        gt = gpool.tile([P, chunk, dh], bf, tag="g")
                        nc.gpsimd.dma_gather(
                            out_ap=gt[:, :k, :],
                            in_ap=st["view"],
                            idxs_ap=st["gidx"][:, c0 * 8:(c0 + k) * 8],
                            num_idxs=k * P,
                            num_idxs_reg=k * P,
                            elem_size=dh,
                            queue_num=st["q"] % 4,
                        )
                        mt = mpool.tile([P, chunk * P], bf, tag="m")
                        nc.sync.dma_start(mt[:, :k * P],
                                          st["mp"][:, c0 * P:(c0 + k) * P])
                        st["gcur"], st["mcur"], st["gc0"] = gt, mt, c0
                    j = t - st["gc0"]
                    return st["mcur"][:, j * P:(j + 1) * P], st["gcur"][:, j, :]

                for b in range(nblk):
                    ps = pspool.tile([P, dh], f32, tag="psa")
                    first = True
                    for st in streams:
                        ts = st["tstart"]
                        for t in range(int(ts[b]), int(ts[b + 1])):
                            m_ap, g_ap = fetch(st, t)
                            nc.tensor.matmul(ps[:], m_ap, g_ap,
                                             start=first, stop=False)
                            first = False
                    # self-loop diagonal, then bias, then relu out of PSUM
                    nc.tensor.matmul(ps[:], dg_sb[:, bsl(b)], hpre_sb[:, bsl(b)],
                                     start=first, stop=False)
                    nc.tensor.matmul(ps[:], ones_sb[:], b_sb[:],
                                     start=False, stop=True)
                    nc.scalar.activation(hout_sb[:, bsl(b)], ps[:], AF.Relu)

            # ---- layer 1 ----
            transform(xT_sb, w1_sb, 0)
            h1_sb = bigpool.tile([P, nloc], f32, tag="hout")
            aggregate(0, b1_sb, h1_sb)

            # ---- layer 2 ----
            h1T_sb = bigpool.tile([P, nloc], bf, tag="lhsT")
            for b in range(nblk):
                ps = pspool.tile([P, P], f32, tag="pst")
                nc.tensor.transpose(ps[:], h1_sb[:, bsl(b)], ident_sb[:])
                nc.vector.tensor_copy(h1T_sb[:, bsl(b)], ps[:])
            transform(h1T_sb, w2_sb, 1)
            h2_sb = bigpool.tile([P, nloc], f32, tag="hout")
            aggregate(1, b2_sb, h2_sb)

            # ---- pooling: sums^T[f, g] over blocks ----
            psp = pspool1.tile([P, g_], f32, tag="pool")
            for b in range(nblk):
                nc.tensor.matmul(psp[:], h2_sb[:, bsl(b)], pm_sb[:, bsl(b, g_)],
                                 start=(b == 0), stop=(b == nblk - 1))
            pool_sb = spool.tile([dh, g_], f32)
            nc.vector.tensor_copy(pool_sb[:], psp[:])
            nc.gpsimd.dma_start(arin[:], pool_sb[:])
            nc.gpsimd.collective_compute(
                "AllReduce", mybir.AluOpType.add, replica_groups=rg,
                ins=[arin[:]], outs=[arout[:]])
            mean_sb = spool.tile([dh, g_], f32)
            nc.sync.dma_start(mean_sb[:], arout[:])

            # ---- final linear + bias ----
            psg = pspool1.tile([g_, do], f32, tag="fin")
            nc.tensor.matmul(psg[:], mean_sb[:], wl_sb[:], start=True, stop=False)
            nc.tensor.matmul(psg[:], onesf_sb[:, :g_], bl_sb[:],
                             start=False, stop=True)
            g_sb = spool.tile([g_, do], f32)
            nc.vector.tensor_copy(g_sb[:], psg[:])

            # ---- L2 normalize rows ----
            sq_sb = spool.tile([g_, do], f32)
            s_sb = spool.tile([g_, 1], f32)
            nrm_sb = spool.tile([g_, 1], f32)
            inv_sb = spool.tile([g_, 1], f32)
            o_sb = spool.tile([g_, do], f32)
            nc.vector.tensor_mul(sq_sb[:], g_sb[:], g_sb[:])
            nc.vector.tensor_reduce(s_sb[:], sq_sb[:],
                                    axis=mybir.AxisListType.X, op=ALU.add)
            nc.scalar.sqrt(nrm_sb[:], s_sb[:])
            nc.vector.tensor_scalar_max(nrm_sb[:], nrm_sb[:], 1e-12)
            nc.vector.reciprocal(inv_sb[:], nrm_sb[:])
            nc.vector.tensor_scalar_mul(o_sb[:], g_sb[:], inv_sb[:, :1])
            nc.sync.dma_start(out_p[:], o_sb[:])

    nc.compile()
    return nc


_CACHE = {}
_LAST_EXEC_NS = None


def _run(cfg, x, W1, b1, W2, b2, Wl, bl, edge_index, batch, trace=False):
    import ml_dtypes
    from concourse.bass_utils import run_bass_kernel_spmd
    bfd = ml_dtypes.bfloat16

    pre = preprocess(cfg, x, edge_index, batch)
    key = (cfg.n_nodes, cfg.nloc, tuple(pre["ttot"]),
           tuple(tuple(T.tolist()) for T in pre["T"]))
    if key not in _CACHE:
        _CACHE[key] = build(cfg, pre["T"], pre["ttot"])
    nc = _CACHE[key]

    in_maps = []
    for c in range(cfg.n_cores):
        m = {}
        for q in range(cfg.nq):
            m[f"gidx{q}"] = np.ascontiguousarray(pre[f"gidx{q}"][c])
            m[f"msel{q}"] = np.ascontiguousarray(pre[f"msel{q}"][c])
        m.update({
            "xT": np.ascontiguousarray(pre["xT"][c]),
            "pm": np.ascontiguousarray(pre["pm"][c]),
            "dg": np.ascontiguousarray(pre["dg"][c]),
            "W1": np.asarray(W1, np.float32).astype(bfd),
            "W2": np.asarray(W2, np.float32).astype(bfd),
            "Wl": np.asarray(Wl, np.float32),
            "b1": np.asarray(b1, np.float32).astype(bfd).reshape(1, -1),
            "b2": np.asarray(b2, np.float32).astype(bfd).reshape(1, -1),
            "bl": np.asarray(bl, np.float32).reshape(1, -1),
        })
        in_maps.append(m)
    res = run_bass_kernel_spmd(nc, in_maps, list(range(cfg.n_cores)),
                               trace=trace)
    global _LAST_EXEC_NS
    _LAST_EXEC_NS = res.exec_time_ns
    return np.asarray(res.results[0]["out"], np.float32)


def kernel(x, W1, b1, W2, b2, Wl, bl, edge_index, batch):
    cfg = GCNConfig()
    return _run(cfg, x, W1, b1, W2, b2, Wl, bl, edge_index, batch)

